# revision 47
# baseline (speedup 1.0000x reference)
"""Trainium2 Bass kernel for nn_CFCEncoder (3-layer CfC RNN encoder), v3.

Transposed (feature-major) formulation:
  - Data-parallel over batch B=512 across 8 cores (64 rows/core); weights
    replicated; K=64-step recurrence local per core.
  - All state kept as (feature, batch) tiles: weights are the PE-stationary
    operand (full 128-wide), batch=64 is the moving dim. fp16 operands keep
    1.0 cycles/row at moving=64 (f32r would take a 4x penalty), and the
    blend output feeds the next matmul directly - no per-step transposes.
  - x-projections are folded into the same psum accumulation group as the
    recurrent matmuls (one group per psum bank per 2-step epoch): the gate
    pre-activations materialize in PSUM with no DVE add, no SBUF xa buffer.
  - Gate banks per 2-step epoch: ff1 (128,4,2,64), ff2, t -> 6 banks with
    double buffering; L1 uses 1 bank; L2's two step-parities share 1 bank.
  - ACT does tanh/sigmoid straight out of PSUM into fp16 SBUF; DVE blends
    in fp16 (2-4x packed throughput); the blend result IS h(t) for the
    next matmuls.

v3 over v2 (384us -> ~360us):
  - L0/L1 recurrent-block weights stored fp8-e3m4 at a power-of-2 scale
    (x-blocks f16 at the same scale, ACT un-scales on the PSUM read):
    mixed e3m4-stationary x f16-moving matmuls run at full rate and the
    fp8 LDWEIGHTS stream (FWL 4B/cycle) stops competing with the matmul
    stream. rel_err ~8.3e-3 (vs 1.1e-3 all-f16, tolerance 2e-2).
  - L2 emitted one step late with both parities in one PSUM bank, and each
    block's x-matmuls emitted during the previous odd step right after the
    L1 matmuls: cold filler sits in the PE queue at the fresh-blend walls
    (NX dispatches in order; a hot semaphore wait drains the engine queue).
  - L1 blend merged across halves (h1 has a step of slack): one tanh +
    one sigmoid + 3 DVE ops instead of 3 ACT + 6 DVE.
  - Startup: first xt block split into its own DMA; w0x chunk 0 config
    issued first so compute starts as soon as the runtime bootstrap ends.
"""

import os
import sys

for _p in ("/root/.axon_site", "/root/.axon_site/_ro/trn_rl_repo",
           "/root/.axon_site/_ro/pypackages", "/opt/trn_rl_repo"):
    if os.path.isdir(_p) and _p not in sys.path:
        sys.path.append(_p)

import numpy as np

NC = 8          # cores
B = 512         # batch
KT = 64         # timesteps
SENS = 768      # sensory features
H = [512, 256, 64]
BC = B // NC    # 64 batch rows per core
NBLK = KT // 2  # 32 two-step blocks
BLK_PER_DMA = 4  # 8 steps per DMA chunk
NXCH = NBLK // BLK_PER_DMA


def split_excess_waits(nc, mybir, limit=1):
    """walrus in this toolchain rejects >1 sem wait on one instruction
    (CTRL struct). Hoist excess waits onto preceding NoOps on the same
    engine (same-engine program order preserves semantics)."""
    cnt = 0
    for fn in nc.m.functions:
        for bb in fn.blocks:
            new_insts = []
            for inst in bb.instructions:
                si = inst.sync_info
                if si is not None and si.on_wait and len(si.on_wait) > limit:
                    waits = list(si.on_wait)
                    excess, keep = waits[:-limit], waits[-limit:]
                    while excess:
                        chunk, excess = excess[:limit], excess[limit:]
                        cnt += 1
                        new_insts.append(mybir.InstNoOp(
                            name=f"I-waitsplit-{cnt}", engine=inst.engine,
                            ins=[], outs=[],
                            sync_info=mybir.SyncInfo(on_wait=chunk, on_update=[])))
                    inst.sync_info = mybir.SyncInfo(
                        on_wait=keep, on_update=list(si.on_update))
                new_insts.append(inst)
            bb.instructions = new_insts


def build_program(s0=64.0, s1=64.0, split_waits=True):
    """s0/s1: power-of-2 scales folded into the L0/L1 weights; the recurrent
    weight blocks are stored fp8-e3m4 (x-blocks f16) and ACT un-scales on the
    PSUM read. fp8 weights halve the LDWEIGHTS stream, which is co-saturated
    with the matmul stream on this kernel."""
    import concourse.bass as bass
    import concourse.tile as tile
    import concourse.mybir as mybir
    from concourse.masks import make_identity

    f32 = mybir.dt.float32
    f16 = mybir.dt.float16
    f8 = mybir.dt.float8e3
    Tanh = mybir.ActivationFunctionType.Tanh
    Sigm = mybir.ActivationFunctionType.Sigmoid

    nc = bass.Bass("TRN2", target_bir_lowering=False, debug=False, num_devices=NC)

    # xt: [partition(128), 2-step block(32), contract chunk(6), parity(2), batch(64)]
    xt_d = nc.dram_tensor("xt", [128, NBLK, 6, 2, BC], f16, kind="ExternalInput").ap()
    w0x_d = nc.dram_tensor("w0x", [768, 1536], f16, kind="ExternalInput").ap()
    w0h_d = nc.dram_tensor("w0h", [512, 1536], f8, kind="ExternalInput").ap()
    w1x_d = nc.dram_tensor("w1x", [512, 768], f16, kind="ExternalInput").ap()
    w1h_d = nc.dram_tensor("w1h", [256, 768], f8, kind="ExternalInput").ap()
    w2_d = nc.dram_tensor("w2", [320, 192], f16, kind="ExternalInput").ap()
    out_d = nc.dram_tensor("out", [BC, H[2]], f32, kind="ExternalOutput").ap()

    with tile.TileContext(nc) as tc:
        with tc.tile_pool(name="pw", bufs=1) as pw, \
             tc.tile_pool(name="pxt", bufs=3) as pxt, \
             tc.tile_pool(name="pact", bufs=2) as pact, \
             tc.tile_pool(name="pblend", bufs=2) as pblend, \
             tc.tile_pool(name="ph", bufs=2) as ph, \
             tc.tile_pool(name="pmisc", bufs=1) as pmisc, \
             tc.tile_pool(name="psA", bufs=2, space="PSUM") as psA, \
             tc.tile_pool(name="psB", bufs=2, space="PSUM") as psB, \
             tc.tile_pool(name="psC", bufs=2, space="PSUM") as psC, \
             tc.tile_pool(name="ps1", bufs=1, space="PSUM") as ps1, \
             tc.tile_pool(name="ps2", bufs=1, space="PSUM") as ps2:

            # ---- resident weights ----
            w0 = [pw.tile([128, 1536], f16, tag=f"w0_{k}", name=f"w0_{k}")
                  for k in range(6)]
            w0h = [pw.tile([128, 1536], f8, tag=f"w0h_{k}", name=f"w0h_{k}")
                   for k in range(4)]
            w1 = [pw.tile([128, 768], f16, tag=f"w1_{k}", name=f"w1_{k}")
                  for k in range(4)]
            w1h = [pw.tile([128, 768], f8, tag=f"w1h_{k}", name=f"w1h_{k}")
                   for k in range(2)]
            w2 = [pw.tile([p, 192], f16, tag=f"w2_{k}", name=f"w2_{k}")
                  for k, p in enumerate((128, 128, 64))]
            xch = {}

            def fetch_chunk(ci, split_first=False):
                # xt streams on the ACT HWDGE queue, parallel to the weight
                # DMAs on the SP/DVE/Pool queues, so compute starts early.
                t = pxt.tile([128, BLK_PER_DMA, 6, 2, BC], f16, tag="xt")
                lo = ci * BLK_PER_DMA
                if split_first:
                    # first block lands in ~1/4 the time: first x-matmuls
                    # only need block 0 + w0x chunk 0.
                    nc.scalar.dma_start(t[:, 0:1], xt_d[:, lo:lo + 1])
                    nc.scalar.dma_start(t[:, 1:], xt_d[:, lo + 1:lo + BLK_PER_DMA])
                else:
                    nc.scalar.dma_start(t[:], xt_d[:, lo:lo + BLK_PER_DMA])
                xch[ci] = t

            nc.sync.dma_start(w0[0][:], w0x_d[0:128, :])
            fetch_chunk(0, split_first=True)
            for k in range(1, 6):
                nc.sync.dma_start(w0[k][:], w0x_d[k * 128:(k + 1) * 128, :])
            for k in range(4):
                nc.scalar.dma_start(w0h[k][:], w0h_d[k * 128:(k + 1) * 128, :])
            for k in range(4):
                nc.sync.dma_start(w1[k][:], w1x_d[k * 128:(k + 1) * 128, :])
            for k in range(2):
                nc.sync.dma_start(w1h[k][:], w1h_d[k * 128:(k + 1) * 128, :])
            for k in range(3):
                nc.sync.dma_start(w2[k][:], w2_d[k * 128:k * 128 + w2[k].shape[0], :])
            fetch_chunk(1)

            ident = pmisc.tile([64, 64], f32, tag="ident")
            make_identity(nc, ident[:])

            h0a = h0b = h2 = None
            banks = {}

            def emit_x(i):
                # 2-step gate banks + x-part matmuls for steps (2i, 2i+1).
                # Emitted during step 2i-1 right after the L1 matmuls, so the
                # x filler sits in the PE queue exactly where the fresh-blend
                # dependency walls (L1-x, next L0-rec) would otherwise stall.
                xcur = xch[i // BLK_PER_DMA]
                lb = i % BLK_PER_DMA
                ba = psA.tile([128, 4, 2, BC], f32, tag="pa")
                bb = psB.tile([128, 4, 2, BC], f32, tag="pb")
                bcc = psC.tile([128, 4, 2, BC], f32, tag="pc")
                banks[i] = (ba, bb, bcc)
                for k in range(6):
                    for bank, gofs in ((ba, 0), (bb, 512), (bcc, 1024)):
                        for c in range(4):
                            nc.tensor.matmul(
                                bank[:, c, :, :],
                                w0[k][:, gofs + 128 * c:gofs + 128 * c + 128],
                                xcur[:, lb, k, :, :],
                                start=(c == 0 and k == 0), stop=False,
                                skip_group_check=True)

            p2full = ps2.tile([64, 2, 3, BC], f32, tag="p2")
            h1ring = pmisc.tile([128, 2, 2, BC], f16, tag="h1ring")

            def emit_l2(tp, h2p, last):
                p2 = p2full[:, (tp + 1) % 2]
                if tp > 0:
                    for g in range(3):
                        nc.tensor.matmul(
                            p2[:, g, :], w2[2][:, 64 * g:64 * g + 64], h2p[:],
                            start=(g == 0), stop=False, skip_group_check=True)
                for j in range(2):
                    for g in range(3):
                        nc.tensor.matmul(
                            p2[:, g, :], w2[j][:, 64 * g:64 * g + 64],
                            h1ring[:, j, tp % 2, :],
                            start=(tp == 0 and j == 0 and g == 0),
                            stop=(j == 1 and g == 2),
                            skip_group_check=True)
                T2 = pact.tile([64, 2, BC], f16, tag="T2")
                nc.scalar.activation(T2[:], p2[:, 0:2, :], Tanh)
                S2 = pact.tile([64, BC], f16, tag="S2")
                nc.scalar.activation(S2[:], p2[:, 2, :], Sigm)
                d2 = pblend.tile([64, BC], f16, tag="d2")
                nc.vector.tensor_sub(d2[:], T2[:, 1, :], T2[:, 0, :])
                e2 = pblend.tile([64, BC], f16, tag="e2")
                nc.vector.tensor_mul(e2[:], d2[:], S2[:])
                h2n = ph.tile([64, BC], f32 if last else f16,
                              tag="h2f" if last else "h2")
                nc.vector.tensor_add(h2n[:], T2[:, 0, :], e2[:])
                return h2n

            emit_x(0)
            for t_step in range(KT):
                blk, par = t_step // 2, t_step % 2

                if par == 0:
                    if blk % BLK_PER_DMA == 0:
                        ci = blk // BLK_PER_DMA + 1
                        if 1 < ci < NXCH:
                            fetch_chunk(ci)
                    pa, pb, pc = banks[blk]

                # ---- L0 recurrent part (emitted half-first so chunk-01
                # activations unlock before half 2's matmuls finish) ----
                if t_step > 0:
                    h0prev = (h0a, h0a, h0b, h0b)
                    for half in (0, 1):
                        for bi, (bank, gofs) in enumerate(
                                ((pa, 0), (pb, 512), (pc, 1024))):
                            for c in (2 * half, 2 * half + 1):
                                for j in range(4):
                                    nc.tensor.matmul(
                                        bank[:, c, par, :],
                                        w0h[j][:, gofs + 128 * c:gofs + 128 * c + 128],
                                        h0prev[j][:, j % 2, :],
                                        start=False,
                                        stop=(half == 1 and bi == 2
                                              and c == 3 and j == 3),
                                        skip_group_check=True)

                # ---- L0 activations + blend, split by chunk halves ----
                h0n = []
                for half, sl in ((0, slice(0, 2)), (1, slice(2, 4))):
                    Ta = pact.tile([128, 2, BC], f16, tag=f"T0a{half}")
                    nc.scalar.activation(Ta[:], pa[:, sl, par, :], Tanh, scale=1.0 / s0)
                    Tb = pact.tile([128, 2, BC], f16, tag=f"T0b{half}")
                    nc.scalar.activation(Tb[:], pb[:, sl, par, :], Tanh, scale=1.0 / s0)
                    S = pact.tile([128, 2, BC], f16, tag=f"S0{half}")
                    nc.scalar.activation(S[:], pc[:, sl, par, :], Sigm, scale=1.0 / s0)
                    dd = pblend.tile([128, 2, BC], f16, tag=f"d0{half}")
                    nc.vector.tensor_sub(dd[:], Tb[:], Ta[:])
                    ee = pblend.tile([128, 2, BC], f16, tag=f"e0{half}")
                    nc.vector.tensor_mul(ee[:], dd[:], S[:])
                    hh = ph.tile([128, 2, BC], f16, tag=f"h0{half}")
                    nc.vector.tensor_add(hh[:], Ta[:], ee[:])
                    h0n.append(hh)

                # ---- L1 ----  slot order [f1c0, f2c0, f1c1, f2c1, t0, t1];
                # emission s-order [0,1,4,5,2,3] so tanh-a, sigm, tanh-b
                # unlock in that order.
                p1 = ps1.tile([128, 6, BC], f32, tag="p1")
                SORD = (0, 1, 4, 5, 2, 3)
                first_l1 = [True]

                def l1_mm(s, wtile, rhs, stop=False):
                    nc.tensor.matmul(
                        p1[:, s, :], wtile[:, 128 * s:128 * s + 128], rhs,
                        start=first_l1[0], stop=stop, skip_group_check=True)
                    first_l1[0] = False

                if t_step > 0:
                    for s in SORD:
                        for j in range(2):
                            l1_mm(s, w1h[j], h1ring[:, j, (t_step - 1) % 2, :])
                for jh, hhalf in ((0, 0), (1, 0), (2, 1), (3, 1)):
                    lastj = (jh == 3)
                    for s in SORD:
                        l1_mm(s, w1[jh], h0n[hhalf][:, jh % 2, :],
                              stop=(lastj and s == 3))

                # L1 blend, merged across halves (h1 has a full step of
                # slack, so fewer/bigger ACT+DVE ops is a pure win).
                # T1 viewed as (half, gate): p1 slots [f1c0, f2c0, f1c1, f2c1]
                T1 = pact.tile([128, 2, 2, BC], f16, tag="T1")
                nc.scalar.activation(T1[:], p1[:, 0:4, :], Tanh, scale=1.0 / s1)
                S1 = pact.tile([128, 2, BC], f16, tag="S1")
                nc.scalar.activation(S1[:], p1[:, 4:6, :], Sigm, scale=1.0 / s1)
                d1 = pblend.tile([128, 2, BC], f16, tag="d1")
                nc.vector.tensor_sub(d1[:], T1[:, :, 1, :], T1[:, :, 0, :])
                e1 = pblend.tile([128, 2, BC], f16, tag="e1")
                nc.vector.tensor_mul(e1[:], d1[:], S1[:])
                nc.vector.tensor_add(h1ring[:, :, par, :], T1[:, :, 0, :], e1[:])

                # next block's x-part, queued as filler behind the walls
                if par == 1 and blk + 1 < NBLK:
                    emit_x(blk + 1)

                # ---- L2, one step delayed ----
                if t_step > 0:
                    h2 = emit_l2(t_step - 1, h2, last=False)

                h0a, h0b = h0n

                if par == 1:
                    banks.pop(blk, None)

            h2 = emit_l2(KT - 1, h2, last=True)

            # ---- output: transpose (hid, batch) -> (batch, hid), DMA ----
            pt = ps2.tile([64, 64], f32, tag="p2")
            nc.tensor.transpose(pt[:], h2[:], ident[:])
            osb = pmisc.tile([64, 64], f32, tag="osb")
            nc.scalar.copy(osb[:], pt[:])
            nc.sync.dma_start(out_d[:], osb[:])

    if split_waits:
        import concourse.mybir as mybir2
        split_excess_waits(nc, mybir2)
    return nc


def _pow2_scale(w, maxv=14.0):
    m = float(np.abs(w).max())
    if m == 0.0:
        return 1.0
    return float(2.0 ** np.floor(np.log2(maxv / m)))


def prep_inputs(base_expanded_seq, visual_seq, weights):
    """weights: dict l{li}_{name} -> np.ndarray. Returns (maps, s0, s1):
    per-core input maps plus the L0/L1 weight scales. The recurrent blocks
    of w0/w1 are stored fp8-e3m4 at scale s; the x blocks f16 at the same
    scale so the shared PSUM accumulation stays consistent."""
    import ml_dtypes

    X = np.concatenate(
        [np.asarray(base_expanded_seq, np.float32),
         np.asarray(visual_seq, np.float32)], axis=-1)       # (B, K, 768)

    wmats = []
    for li in range(3):
        g = lambda n: np.asarray(weights[f"l{li}_{n}"], np.float32)
        mask = g("mask")
        f1, f2, tg = g("ff1_w") * mask, g("ff2_w") * mask, g("ta_w") + g("tb_w")
        wcat = np.concatenate([f1, f2, tg], axis=0)          # (3h, cat)
        wmats.append(np.ascontiguousarray(wcat.T))           # (cat, 3h)

    w0f = wmats[0]                                           # (1280, 1536)
    s0 = _pow2_scale(w0f[768:])
    w0x = (w0f[:768] * s0).astype(np.float16)                # (768, 1536)
    w0h = (w0f[768:] * s0).astype(ml_dtypes.float8_e3m4)     # (512, 1536)
    # L1 gate slots interleaved [f1c0, f2c0, f1c1, f2c1, t0, t1] so the
    # kernel's half-blends read contiguous psum slots.
    w1f = wmats[1]                                           # (768, 768)
    w1f = np.concatenate(
        [w1f[:, 0:128], w1f[:, 256:384], w1f[:, 128:256],
         w1f[:, 384:512], w1f[:, 512:768]], axis=1)
    s1 = _pow2_scale(w1f[512:])
    w1x = (w1f[:512] * s1).astype(np.float16)                # (512, 768)
    w1h = (w1f[512:] * s1).astype(ml_dtypes.float8_e3m4)     # (256, 768)
    w2 = wmats[2].astype(np.float16)                         # (320, 192)

    maps = []
    for c in range(NC):
        Xc = X[c * BC:(c + 1) * BC]                          # (64, K, 768)
        arr = Xc.transpose(2, 1, 0)                          # (768, K, 64)
        arr = arr.reshape(6, 128, NBLK, 2, BC)               # (k, p, blk, par, b)
        xt = np.ascontiguousarray(arr.transpose(1, 2, 0, 3, 4)).astype(np.float16)
        maps.append({"xt": xt, "w0x": w0x, "w0h": w0h,
                     "w1x": w1x, "w1h": w1h, "w2": w2})
    return maps, s0, s1


_CACHE = {}


def run_on_device(maps, s0, s1, trace=False):
    from concourse.bass_utils import run_bass_kernel_spmd
    key = ("nc", s0, s1)
    if key not in _CACHE:
        _CACHE[key] = build_program(s0=s0, s1=s1)
    nc = _CACHE[key]
    kw = {}
    if trace:
        kw = dict(trace=True, trace_cores=[0])
    return run_bass_kernel_spmd(nc, maps, list(range(NC)), **kw)


def kernel(**inputs):
    base = inputs["base_expanded_seq"]
    vis = inputs["visual_seq"]
    maps, s0, s1 = prep_inputs(base, vis, inputs)
    res = run_on_device(maps, s0, s1, trace=False)
    out = np.concatenate(
        [res.results[c]["out"] for c in range(NC)], axis=0)  # (512, 64)
    return out.astype(np.float32)

